# revision 15
# baseline (speedup 1.0000x reference)
"""AttnBlock (GroupNorm + single-head self-attention + residual) on 8 TRN2
NeuronCores.

Reference computation (per image b of 4, tokens N=64*64=4096, C=512):
    hn  = GroupNorm(x)  (32 groups, eps 1e-6, affine)
    q,k,v = hn @ wq + bq, ...
    attn = softmax(q @ k.T / sqrt(C)); out = attn @ v
    y   = x + out @ wo + bo

Sharding: one NeuronCore per (image, half): core 2b+h computes attention
rows [h*2048, (h+1)*2048) of image b. Each core redundantly computes
GroupNorm stats and full-image K/V (cheap vs. cross-core collectives) and
its own 2048 query rows. No inter-core communication.

Per-core layout: everything feature-major ([C, tokens]) so every matmul
contraction sits on the partition axis; the final projection naturally
returns to row-major. The host pre-transposes/casts x to bf16
feature-major per core (shard prep) and passes the residual rows in f32.

Device pipeline:
  1. GroupNorm statistics via bn_stats/bn_aggr on the raw bf16 x
     (feature-major; per-channel over tokens, then group-combined with a
     block-diagonal averaging matmul).
  2. The normalization hn = x*A + B is FOLDED INTO THE QKV WEIGHTS:
     W' = A∘W (row scale), b' = b + B@W. The projections then consume the
     raw x tiles directly - no normalize pass on the critical path.
  3. Attention: scoresT = kT^T q (feature-major both sides), exp on the
     Scalar engine without max subtraction (scores provably in [-2,2] for
     unit-normalized inputs), softmax denominator accumulated on the
     Vector engine, attn@v and output projection on TensorE, with the
     1/denominator applied per query row in the epilogue (softmax
     normalization commutes with the linear attn@v and output proj).
Compute dtype: bf16 operands, f32 PSUM accumulation.
"""

import sys

if "/opt/trn_rl_repo" not in sys.path:
    sys.path.insert(0, "/opt/trn_rl_repo")

import numpy as np
import ml_dtypes

import concourse.bass as bass
import concourse.tile as tile
from concourse import bacc, mybir
from concourse.bass_utils import run_bass_kernel_spmd

F32 = mybir.dt.float32
BF16 = mybir.dt.bfloat16

B, H, W, C = 4, 64, 64, 512
N_TOK = H * W            # tokens per image
NQ = N_TOK // 2          # query rows per core
G = 32                   # groups
GS = C // G              # channels per group (16)
EPS = 1e-6
SCALE = float(C) ** -0.5
CT = C // 128            # channel tiles (4)
JT = N_TOK // 128        # token tiles (32)
IB = NQ // 512           # query i-blocks (4)

_CACHE = {}


def _build():
    nc = bacc.Bacc("TRN2", target_bir_lowering=False)

    xt_e = nc.dram_tensor("xt", [C, N_TOK], BF16, kind="ExternalInput")
    xr_e = nc.dram_tensor("xr", [NQ, C], F32, kind="ExternalInput")
    w_e = {
        n: nc.dram_tensor(n, [C, C], BF16, kind="ExternalInput")
        for n in ("wq", "wk", "wv", "wo")
    }
    b_e = {
        n: nc.dram_tensor(n, [C], F32, kind="ExternalInput")
        for n in ("bq", "bk", "bv")
    }
    gs_e = nc.dram_tensor("gsc", [C], F32, kind="ExternalInput")
    gb_e = nc.dram_tensor("gbi", [C], F32, kind="ExternalInput")
    gm_e = nc.dram_tensor("gmat", [128, 128], F32, kind="ExternalInput")
    out_e = nc.dram_tensor("out", [NQ, C], F32, kind="ExternalOutput")

    def col(e):  # [C] dram -> [C,1] view for partition-major loads
        return e.ap().rearrange("(a b) -> a b", b=1)

    with tile.TileContext(nc) as tc:
        with (
            tc.tile_pool(name="const", bufs=1) as const,
            tc.tile_pool(name="big", bufs=1) as big,
            tc.tile_pool(name="stat", bufs=1) as stat,
            tc.tile_pool(name="ework", bufs=4) as ework,
            tc.tile_pool(name="attw", bufs=8) as attw,
            tc.tile_pool(name="owork", bufs=4) as owork,
            tc.tile_pool(name="xrw", bufs=3) as xrw,
            tc.tile_pool(name="rdenw", bufs=8) as rdenw,
        ):
            # ---- x feature-major (bf16) ----
            # whole-tile contiguous DMAs (1 MB each; strided sub-chunk loads
            # are an order of magnitude slower), issued before the weight
            # loads so the stats-critical data arrives first
            xT = []
            for k in range(CT):
                t = big.tile([128, N_TOK], BF16, tag=f"xT{k}", name=f"xT{k}")
                nc.sync.dma_start(out=t, in_=xt_e.ap()[k * 128:(k + 1) * 128, :])
                xT.append(t)

            # ---- weights / constants ----
            wsb = {}
            for n in ("wq", "wk", "wv", "wo"):
                wsb[n] = []
                for k in range(CT):
                    t = const.tile([128, C], BF16, tag=f"w_{n}_{k}")
                    nc.sync.dma_start(out=t, in_=w_e[n].ap()[k * 128:(k + 1) * 128, :])
                    wsb[n].append(t)
            bsb = {}
            for n in ("bq", "bk"):
                bsb[n] = []
                for m in range(CT):
                    t = const.tile([128, 1], F32, tag=f"b_{n}_{m}")
                    nc.sync.dma_start(out=t, in_=col(b_e[n])[m * 128:(m + 1) * 128, :])
                    bsb[n].append(t)
            bvb = const.tile([128, C], F32, tag="bvb")
            nc.sync.dma_start(
                out=bvb,
                in_=bass.AP(tensor=b_e["bv"], offset=0, ap=[[0, 128], [1, C]]),
            )
            gssb, gbsb = [], []
            for m in range(CT):
                t = const.tile([128, 1], F32, tag=f"gs_{m}")
                nc.sync.dma_start(out=t, in_=col(gs_e)[m * 128:(m + 1) * 128, :])
                gssb.append(t)
                t = const.tile([128, 1], F32, tag=f"gb_{m}")
                nc.sync.dma_start(out=t, in_=col(gb_e)[m * 128:(m + 1) * 128, :])
                gbsb.append(t)
            gm_sb = const.tile([128, 128], F32, tag="gmat")
            nc.sync.dma_start(out=gm_sb, in_=gm_e.ap())
            ones_bcol = const.tile([1, 128], F32, tag="ones_bcol")
            nc.vector.memset(ones_bcol, 1.0)
            onef = const.tile([1, 1], F32, tag="onef")
            nc.vector.memset(onef, 1.0)
            onesf_col = const.tile([128, 1], F32, tag="onesf_col")
            nc.vector.memset(onesf_col, 1.0)
            epst = const.tile([128, 1], F32, tag="epst")
            nc.vector.memset(epst, EPS)

            # ---- GroupNorm stats + weight folding ----
            with tc.tile_pool(name="ps_misc", bufs=2, space="PSUM") as psm:
                Af, Bbf = [], []   # A (f32 [128,1]); B cast to bf16 for matmuls
                for k in range(CT):
                    stats = stat.tile([128, 8, 6], F32, tag=f"st{k}")
                    for ch in range(8):
                        nc.vector.bn_stats(
                            out=stats[:, ch, :],
                            in_=xT[k][:, ch * 512:(ch + 1) * 512],
                        )
                    mv = stat.tile([128, 2], F32, tag=f"mv{k}")
                    nc.vector.bn_aggr(out=mv, in_=stats)
                    # sm = (mean, var + mean^2) per channel
                    sm = stat.tile([128, 2], F32, tag=f"sm{k}")
                    nc.vector.tensor_copy(out=sm[:, 0:1], in_=mv[:, 0:1])
                    nc.vector.tensor_mul(out=sm[:, 1:2], in0=mv[:, 0:1], in1=mv[:, 0:1])
                    nc.vector.tensor_add(out=sm[:, 1:2], in0=sm[:, 1:2], in1=mv[:, 1:2])
                    # group-average via block-diagonal (1/GS) matrix
                    gps = psm.tile([128, 2], F32, tag="gps")
                    nc.tensor.matmul(gps, gm_sb, sm, start=True, stop=True)
                    gsb = stat.tile([128, 2], F32, tag=f"gsb{k}")
                    nc.vector.tensor_copy(out=gsb, in_=gps)
                    # var_g = E_g[x^2]-mean_g^2; A = rstd*scale; B = bias-mean_g*A
                    msq = stat.tile([128, 1], F32, tag=f"msq{k}")
                    nc.vector.tensor_mul(out=msq, in0=gsb[:, 0:1], in1=gsb[:, 0:1])
                    varg = stat.tile([128, 1], F32, tag=f"vg{k}")
                    nc.vector.tensor_sub(out=varg, in0=gsb[:, 1:2], in1=msq)
                    sd = stat.tile([128, 1], F32, tag=f"sd{k}")
                    nc.scalar.activation(
                        out=sd, in_=varg,
                        func=mybir.ActivationFunctionType.Sqrt,
                        bias=epst, scale=1.0,
                    )
                    rstd = stat.tile([128, 1], F32, tag=f"rs{k}")
                    nc.vector.reciprocal(out=rstd, in_=sd)
                    At = stat.tile([128, 1], F32, tag=f"A{k}")
                    nc.vector.tensor_mul(out=At, in0=rstd, in1=gssb[k])
                    mA = stat.tile([128, 1], F32, tag=f"mA{k}")
                    nc.vector.tensor_mul(out=mA, in0=gsb[:, 0:1], in1=At)
                    Bt = stat.tile([128, 1], F32, tag=f"B{k}")
                    nc.vector.tensor_sub(out=Bt, in0=gbsb[k], in1=mA)
                    Bb = stat.tile([128, 1], BF16, tag=f"Bb{k}")
                    nc.vector.tensor_copy(out=Bb, in_=Bt)
                    Af.append(At)
                    Bbf.append(Bb)

                # fold normalization into weights:
                #   W' = A (row) ∘ W ;  b' = b + B @ W
                # B@W matmuls read the ORIGINAL W (Tile orders them before the
                # in-place row scale below via WAR deps).
                badj = {}
                for n in ("wq", "wk", "wv"):
                    pb = psm.tile([1, 512], F32, tag="pb", name=f"pb_{n}")
                    for k in range(CT):
                        nc.tensor.matmul(
                            pb, Bbf[k], wsb[n][k],
                            start=(k == 0), stop=(k == CT - 1),
                        )
                    bs_ = stat.tile([1, 512], F32, tag=f"badj_{n}")
                    nc.vector.tensor_copy(out=bs_, in_=pb)
                    badj[n] = bs_
                for n in ("wq", "wk", "wv"):
                    for k in range(CT):
                        nc.gpsimd.tensor_scalar_mul(
                            out=wsb[n][k], in0=wsb[n][k], scalar1=Af[k],
                        )
                # transpose b' pieces to per-partition layout for q/k;
                # build broadcast bias for v.
                bqf, bkf = [], []
                for n, dst in (("wq", bqf), ("wk", bkf)):
                    for m in range(CT):
                        pt = psm.tile([128, 1], F32, tag="pt", name=f"pt_{n}{m}")
                        nc.tensor.matmul(
                            pt, badj[n][0:1, m * 128:(m + 1) * 128], onef,
                            start=True, stop=True,
                        )
                        bf = stat.tile([128, 1], F32, tag=f"bf_{n}{m}")
                        base = bsb["bq"][m] if n == "wq" else bsb["bk"][m]
                        nc.vector.tensor_add(out=bf, in0=pt, in1=base)
                        dst.append(bf)
                pvb = psm.tile([128, 512], F32, tag="pvb")
                nc.tensor.matmul(pvb, ones_bcol, badj["wv"], start=True, stop=True)
                nc.vector.tensor_add(out=bvb, in0=pvb, in1=bvb)

            # ---- projections (raw x in, folded weights) ----
            kT = [big.tile([128, N_TOK], BF16, tag=f"kT{m}", name=f"kT{m}")
                  for m in range(CT)]
            qT = [big.tile([128, NQ], BF16, tag=f"qT{m}", name=f"qT{m}")
                  for m in range(CT)]
            v_sb = big.tile([128, JT, C], BF16, tag="v")
            with tc.tile_pool(name="ps_proj", bufs=6, space="PSUM") as psp:
                for m in range(CT):
                    for nt in range(N_TOK // 512):
                        pk = psp.tile([128, 512], F32, tag="p")
                        for k in range(CT):
                            nc.tensor.matmul(
                                pk,
                                wsb["wk"][k][:, m * 128:(m + 1) * 128],
                                xT[k][:, nt * 512:(nt + 1) * 512],
                                start=(k == 0), stop=(k == CT - 1),
                            )
                        nc.vector.tensor_scalar_add(
                            out=kT[m][:, nt * 512:(nt + 1) * 512],
                            in0=pk, scalar1=bkf[m],
                        )
                    for nt in range(NQ // 512):
                        pq = psp.tile([128, 512], F32, tag="p")
                        for k in range(CT):
                            nc.tensor.matmul(
                                pq,
                                wsb["wq"][k][:, m * 128:(m + 1) * 128],
                                xT[k][:, nt * 512:(nt + 1) * 512],
                                start=(k == 0), stop=(k == CT - 1),
                            )
                        nc.vector.tensor_scalar_add(
                            out=qT[m][:, nt * 512:(nt + 1) * 512],
                            in0=pq, scalar1=bqf[m],
                        )
                for jt in range(JT):
                    pv = psp.tile([128, 512], F32, tag="p")
                    for k in range(CT):
                        nc.tensor.matmul(
                            pv,
                            xT[k][:, jt * 128:(jt + 1) * 128],
                            wsb["wv"][k],
                            start=(k == 0), stop=(k == CT - 1),
                        )
                    nc.vector.tensor_add(out=v_sb[:, jt, :], in0=pv, in1=bvb)

            # ---- attention ----
            with (
                tc.tile_pool(name="ps_att", bufs=4, space="PSUM") as psa,
                tc.tile_pool(name="ps_s", bufs=2, space="PSUM") as pss,
                tc.tile_pool(name="ps_o", bufs=1, space="PSUM") as pso,
                tc.tile_pool(name="ps_den", bufs=1, space="PSUM") as psd,
            ):
                for ib in range(IB):
                    att_ps = [psa.tile([128, 512], F32, tag="att", name=f"att_ps{cs}")
                              for cs in range(CT)]
                    dacc = owork.tile([128, 512], F32, tag="dacc")
                    nc.vector.memset(dacc, 0.0)
                    for jt in range(JT):
                        s_ps = pss.tile([128, 512], F32, tag="s")
                        for k in range(CT):
                            nc.tensor.matmul(
                                s_ps,
                                kT[k][:, jt * 128:(jt + 1) * 128],
                                qT[k][:, ib * 512:(ib + 1) * 512],
                                start=(k == 0), stop=(k == CT - 1),
                            )
                        e_t = ework.tile([128, 512], BF16, tag="e")
                        nc.scalar.activation(
                            out=e_t, in_=s_ps,
                            func=mybir.ActivationFunctionType.Exp,
                            scale=SCALE,
                        )
                        for cs in range(CT):
                            nc.tensor.matmul(
                                att_ps[cs],
                                v_sb[:, jt, cs * 128:(cs + 1) * 128],
                                e_t,
                                start=(jt == 0), stop=(jt == JT - 1),
                            )
                        nc.vector.tensor_add(out=dacc, in0=dacc, in1=e_t)
                    # denominator: column sums of dacc (over j partitions)
                    den_ps = psd.tile([1, 512], F32, tag="den")
                    nc.tensor.matmul(den_ps, onesf_col, dacc, start=True, stop=True)
                    attT = []
                    for cs in range(CT):
                        t = attw.tile([128, 512], BF16, tag="attT", name=f"attT{cs}")
                        nc.vector.tensor_copy(out=t, in_=att_ps[cs])
                        attT.append(t)
                    den_sb = owork.tile([1, 512], F32, tag="den_sb")
                    nc.vector.tensor_copy(out=den_sb, in_=den_ps)
                    for it in range(4):
                        row0 = (ib * 4 + it) * 128
                        dT = pso.tile([128, 1], F32, tag="o",
                                      padded_shape=[128, 512], name=f"dT{it}")
                        nc.tensor.matmul(
                            dT, den_sb[0:1, it * 128:(it + 1) * 128], onef,
                            start=True, stop=True,
                        )
                        rden = rdenw.tile([128, 1], F32, tag="rden")
                        nc.vector.reciprocal(out=rden, in_=dT)
                        o_ps = pso.tile([128, 512], F32, tag="o", name=f"o_ps{it}")
                        for cs in range(CT):
                            nc.tensor.matmul(
                                o_ps,
                                attT[cs][:, it * 128:(it + 1) * 128],
                                wsb["wo"][cs],
                                start=(cs == 0), stop=(cs == CT - 1),
                            )
                        xr_t = xrw.tile([128, C], F32, tag="xr")
                        nc.sync.dma_start(
                            out=xr_t, in_=xr_e.ap()[row0:row0 + 128, :]
                        )
                        o_t = owork.tile([128, C], F32, tag="o")
                        nc.vector.scalar_tensor_tensor(
                            out=o_t, in0=o_ps, scalar=rden, in1=xr_t,
                            op0=mybir.AluOpType.mult, op1=mybir.AluOpType.add,
                        )
                        nc.sync.dma_start(
                            out=out_e.ap()[row0:row0 + 128, :], in_=o_t
                        )

    nc.compile()
    return nc


def _get_nc():
    if "nc" not in _CACHE:
        _CACHE["nc"] = _build()
    return _CACHE["nc"]


def kernel(**inputs) -> np.ndarray:
    x = np.asarray(inputs["x"], dtype=np.float32)          # [B,H,W,C]
    gn_scale = np.asarray(inputs["gn_scale"], np.float32)
    gn_bias = np.asarray(inputs["gn_bias"], np.float32)
    ws = {n: np.ascontiguousarray(
        np.asarray(inputs[n], np.float32).astype(ml_dtypes.bfloat16))
        for n in ("wq", "wk", "wv", "wo")}
    bs = {n: np.asarray(inputs[n], np.float32) for n in ("bq", "bk", "bv", "bo")}

    gmat = np.zeros((128, 128), np.float32)
    for g in range(128 // GS):
        gmat[g * GS:(g + 1) * GS, g * GS:(g + 1) * GS] = 1.0 / GS

    xf = x.reshape(B, N_TOK, C)
    in_maps = []
    for core in range(8):
        b, h = divmod(core, 2)
        own = xf[b, h * NQ:(h + 1) * NQ]          # [NQ, C] fp32
        other = xf[b, (1 - h) * NQ:(2 - h) * NQ]
        perm = np.concatenate([own, other], axis=0)        # own half first
        xt = np.ascontiguousarray(perm.T.astype(ml_dtypes.bfloat16))  # [C, N]
        xr = np.ascontiguousarray(own + bs["bo"][None, :])  # residual (+bo)
        in_maps.append({
            "xt": xt,
            "xr": xr,
            "wq": ws["wq"], "wk": ws["wk"], "wv": ws["wv"], "wo": ws["wo"],
            "bq": bs["bq"], "bk": bs["bk"], "bv": bs["bv"],
            "gsc": gn_scale, "gbi": gn_bias,
            "gmat": gmat,
        })

    nc = _get_nc()
    res = run_bass_kernel_spmd(nc, in_maps, core_ids=list(range(8)))

    out = np.empty((B, N_TOK, C), np.float32)
    for core in range(8):
        b, h = divmod(core, 2)
        out[b, h * NQ:(h + 1) * NQ] = res.results[core]["out"]
    return out.reshape(B, H, W, C)


# revision 16
# speedup vs baseline: 1.3294x; 1.3294x over previous
"""AttnBlock (GroupNorm + single-head self-attention + residual) on 8 TRN2
NeuronCores.

Reference computation (per image b of 4, tokens N=64*64=4096, C=512):
    hn  = GroupNorm(x)  (32 groups, eps 1e-6, affine)
    q,k,v = hn @ wq + bq, ...
    attn = softmax(q @ k.T / sqrt(C)); out = attn @ v
    y   = x + out @ wo + bo

Sharding: one NeuronCore per (image, half): core 2b+h computes attention
rows [h*2048, (h+1)*2048) of image b. Each core redundantly computes
GroupNorm stats and full-image K/V (cheap vs. cross-core collectives) and
its own 2048 query rows. No inter-core communication.

Per-core layout: everything feature-major ([C, tokens]) so every matmul
contraction sits on the partition axis; the final projection naturally
returns to row-major. The host pre-transposes/casts x to bf16
feature-major per core (shard prep) and passes the residual rows in f32.

Device pipeline:
  1. GroupNorm statistics via bn_stats/bn_aggr on the raw bf16 x
     (feature-major; per-channel over tokens, then group-combined with a
     block-diagonal averaging matmul).
  2. The normalization hn = x*A + B is FOLDED INTO THE QKV WEIGHTS:
     W' = A∘W (row scale), b' = b + B@W. The projections then consume the
     raw x tiles directly - no normalize pass on the critical path.
  3. Attention: scoresT = kT^T q (feature-major both sides), exp on the
     Scalar engine without max subtraction (scores provably in [-2,2] for
     unit-normalized inputs), softmax denominator accumulated on the
     Vector engine, attn@v and output projection on TensorE, with the
     1/denominator applied per query row in the epilogue (softmax
     normalization commutes with the linear attn@v and output proj).
Compute dtype: bf16 operands, f32 PSUM accumulation.
"""

import sys

if "/opt/trn_rl_repo" not in sys.path:
    sys.path.insert(0, "/opt/trn_rl_repo")

import numpy as np
import ml_dtypes

import concourse.bass as bass
import concourse.tile as tile
from concourse import bacc, mybir
from concourse.bass_utils import run_bass_kernel_spmd

F32 = mybir.dt.float32
BF16 = mybir.dt.bfloat16

B, H, W, C = 4, 64, 64, 512
N_TOK = H * W            # tokens per image
NQ = N_TOK // 2          # query rows per core
G = 32                   # groups
GS = C // G              # channels per group (16)
EPS = 1e-6
SCALE = float(C) ** -0.5
CT = C // 128            # channel tiles (4)
JT = N_TOK // 128        # token tiles (32)
IB = NQ // 512           # query i-blocks (4)

_CACHE = {}


def _build():
    nc = bacc.Bacc("TRN2", target_bir_lowering=False)

    xt_e = nc.dram_tensor("xt", [C, N_TOK], BF16, kind="ExternalInput")
    xr_e = nc.dram_tensor("xr", [NQ, C], F32, kind="ExternalInput")
    w_e = {
        n: nc.dram_tensor(n, [C, C], BF16, kind="ExternalInput")
        for n in ("wq", "wk", "wv", "wo")
    }
    b_e = {
        n: nc.dram_tensor(n, [C], F32, kind="ExternalInput")
        for n in ("bq", "bk", "bv")
    }
    gs_e = nc.dram_tensor("gsc", [C], F32, kind="ExternalInput")
    gb_e = nc.dram_tensor("gbi", [C], F32, kind="ExternalInput")
    gm_e = nc.dram_tensor("gmat", [128, 128], F32, kind="ExternalInput")
    out_e = nc.dram_tensor("out", [NQ, C], F32, kind="ExternalOutput")

    def col(e):  # [C] dram -> [C,1] view for partition-major loads
        return e.ap().rearrange("(a b) -> a b", b=1)

    with tile.TileContext(nc) as tc:
        with (
            tc.tile_pool(name="const", bufs=1) as const,
            tc.tile_pool(name="big", bufs=1) as big,
            tc.tile_pool(name="stat", bufs=1) as stat,
            tc.tile_pool(name="ework", bufs=4) as ework,
            tc.tile_pool(name="attw", bufs=8) as attw,
            tc.tile_pool(name="owork", bufs=4) as owork,
            tc.tile_pool(name="xrw", bufs=3) as xrw,
            tc.tile_pool(name="rdenw", bufs=8) as rdenw,
        ):
            # ---- x feature-major (bf16) ----
            # whole-tile contiguous DMAs (1 MB each; strided sub-chunk loads
            # are an order of magnitude slower), issued before the weight
            # loads so the stats-critical data arrives first
            xT = []
            for k in range(CT):
                t = big.tile([128, N_TOK], BF16, tag=f"xT{k}", name=f"xT{k}")
                nc.sync.dma_start(out=t, in_=xt_e.ap()[k * 128:(k + 1) * 128, :])
                xT.append(t)

            # ---- weights / constants ----
            wsb = {}
            for n in ("wq", "wk", "wv", "wo"):
                wsb[n] = []
                for k in range(CT):
                    t = const.tile([128, C], BF16, tag=f"w_{n}_{k}")
                    nc.sync.dma_start(out=t, in_=w_e[n].ap()[k * 128:(k + 1) * 128, :])
                    wsb[n].append(t)
            bsb = {}
            for n in ("bq", "bk"):
                bsb[n] = []
                for m in range(CT):
                    t = const.tile([128, 1], F32, tag=f"b_{n}_{m}")
                    nc.sync.dma_start(out=t, in_=col(b_e[n])[m * 128:(m + 1) * 128, :])
                    bsb[n].append(t)
            bvb = const.tile([128, C], F32, tag="bvb")
            nc.sync.dma_start(
                out=bvb,
                in_=bass.AP(tensor=b_e["bv"], offset=0, ap=[[0, 128], [1, C]]),
            )
            gssb, gbsb = [], []
            for m in range(CT):
                t = const.tile([128, 1], F32, tag=f"gs_{m}")
                nc.sync.dma_start(out=t, in_=col(gs_e)[m * 128:(m + 1) * 128, :])
                gssb.append(t)
                t = const.tile([128, 1], F32, tag=f"gb_{m}")
                nc.sync.dma_start(out=t, in_=col(gb_e)[m * 128:(m + 1) * 128, :])
                gbsb.append(t)
            gm_sb = const.tile([128, 128], F32, tag="gmat")
            nc.sync.dma_start(out=gm_sb, in_=gm_e.ap())
            ones_bcol = const.tile([1, 128], F32, tag="ones_bcol")
            nc.vector.memset(ones_bcol, 1.0)
            onef = const.tile([1, 1], F32, tag="onef")
            nc.vector.memset(onef, 1.0)
            onesf_col = const.tile([128, 1], F32, tag="onesf_col")
            nc.vector.memset(onesf_col, 1.0)
            epst = const.tile([128, 1], F32, tag="epst")
            nc.vector.memset(epst, EPS)

            # ---- GroupNorm stats + weight folding ----
            with tc.tile_pool(name="ps_misc", bufs=2, space="PSUM") as psm:
                Af, Bbf = [], []   # A (f32 [128,1]); B cast to bf16 for matmuls
                for k in range(CT):
                    stats = stat.tile([128, 8, 6], F32, tag=f"st{k}")
                    for ch in range(8):
                        nc.vector.bn_stats(
                            out=stats[:, ch, :],
                            in_=xT[k][:, ch * 512:(ch + 1) * 512],
                        )
                    mv = stat.tile([128, 2], F32, tag=f"mv{k}")
                    nc.vector.bn_aggr(out=mv, in_=stats)
                    # sm = (mean, var + mean^2) per channel
                    sm = stat.tile([128, 2], F32, tag=f"sm{k}")
                    nc.vector.tensor_copy(out=sm[:, 0:1], in_=mv[:, 0:1])
                    nc.vector.tensor_mul(out=sm[:, 1:2], in0=mv[:, 0:1], in1=mv[:, 0:1])
                    nc.vector.tensor_add(out=sm[:, 1:2], in0=sm[:, 1:2], in1=mv[:, 1:2])
                    # group-average via block-diagonal (1/GS) matrix
                    gps = psm.tile([128, 2], F32, tag="gps")
                    nc.tensor.matmul(gps, gm_sb, sm, start=True, stop=True)
                    gsb = stat.tile([128, 2], F32, tag=f"gsb{k}")
                    nc.vector.tensor_copy(out=gsb, in_=gps)
                    # var_g = E_g[x^2]-mean_g^2; A = rstd*scale; B = bias-mean_g*A
                    msq = stat.tile([128, 1], F32, tag=f"msq{k}")
                    nc.vector.tensor_mul(out=msq, in0=gsb[:, 0:1], in1=gsb[:, 0:1])
                    varg = stat.tile([128, 1], F32, tag=f"vg{k}")
                    nc.vector.tensor_sub(out=varg, in0=gsb[:, 1:2], in1=msq)
                    sd = stat.tile([128, 1], F32, tag=f"sd{k}")
                    nc.scalar.activation(
                        out=sd, in_=varg,
                        func=mybir.ActivationFunctionType.Sqrt,
                        bias=epst, scale=1.0,
                    )
                    rstd = stat.tile([128, 1], F32, tag=f"rs{k}")
                    nc.vector.reciprocal(out=rstd, in_=sd)
                    At = stat.tile([128, 1], F32, tag=f"A{k}")
                    nc.vector.tensor_mul(out=At, in0=rstd, in1=gssb[k])
                    mA = stat.tile([128, 1], F32, tag=f"mA{k}")
                    nc.vector.tensor_mul(out=mA, in0=gsb[:, 0:1], in1=At)
                    Bt = stat.tile([128, 1], F32, tag=f"B{k}")
                    nc.vector.tensor_sub(out=Bt, in0=gbsb[k], in1=mA)
                    Bb = stat.tile([128, 1], BF16, tag=f"Bb{k}")
                    nc.vector.tensor_copy(out=Bb, in_=Bt)
                    Af.append(At)
                    Bbf.append(Bb)

                # fold normalization into weights:
                #   W' = A (row) ∘ W ;  b' = b + B @ W
                # B@W matmuls read the ORIGINAL W (Tile orders them before the
                # in-place row scale below via WAR deps).
                badj = {}
                for n in ("wq", "wk", "wv"):
                    pb = psm.tile([1, 512], F32, tag="pb", name=f"pb_{n}")
                    for k in range(CT):
                        nc.tensor.matmul(
                            pb, Bbf[k], wsb[n][k],
                            start=(k == 0), stop=(k == CT - 1),
                        )
                    bs_ = stat.tile([1, 512], F32, tag=f"badj_{n}")
                    nc.vector.tensor_copy(out=bs_, in_=pb)
                    badj[n] = bs_
                for n in ("wq", "wk", "wv"):
                    for k in range(CT):
                        nc.vector.tensor_scalar_mul(
                            out=wsb[n][k], in0=wsb[n][k], scalar1=Af[k],
                        )
                # transpose b' pieces to per-partition layout for q/k;
                # build broadcast bias for v.
                bqf, bkf = [], []
                for n, dst in (("wq", bqf), ("wk", bkf)):
                    for m in range(CT):
                        pt = psm.tile([128, 1], F32, tag="pt", name=f"pt_{n}{m}")
                        nc.tensor.matmul(
                            pt, badj[n][0:1, m * 128:(m + 1) * 128], onef,
                            start=True, stop=True,
                        )
                        bf = stat.tile([128, 1], F32, tag=f"bf_{n}{m}")
                        base = bsb["bq"][m] if n == "wq" else bsb["bk"][m]
                        nc.vector.tensor_add(out=bf, in0=pt, in1=base)
                        dst.append(bf)
                pvb = psm.tile([128, 512], F32, tag="pvb")
                nc.tensor.matmul(pvb, ones_bcol, badj["wv"], start=True, stop=True)
                nc.vector.tensor_add(out=bvb, in0=pvb, in1=bvb)

            # ---- projections (raw x in, folded weights) ----
            kT = [big.tile([128, N_TOK], BF16, tag=f"kT{m}", name=f"kT{m}")
                  for m in range(CT)]
            qT = [big.tile([128, NQ], BF16, tag=f"qT{m}", name=f"qT{m}")
                  for m in range(CT)]
            v_sb = big.tile([128, JT, C], BF16, tag="v")
            with tc.tile_pool(name="ps_proj", bufs=6, space="PSUM") as psp:
                for m in range(CT):
                    for nt in range(N_TOK // 512):
                        pk = psp.tile([128, 512], F32, tag="p")
                        for k in range(CT):
                            nc.tensor.matmul(
                                pk,
                                wsb["wk"][k][:, m * 128:(m + 1) * 128],
                                xT[k][:, nt * 512:(nt + 1) * 512],
                                start=(k == 0), stop=(k == CT - 1),
                            )
                        nc.vector.tensor_scalar_add(
                            out=kT[m][:, nt * 512:(nt + 1) * 512],
                            in0=pk, scalar1=bkf[m],
                        )
                    for nt in range(NQ // 512):
                        pq = psp.tile([128, 512], F32, tag="p")
                        for k in range(CT):
                            nc.tensor.matmul(
                                pq,
                                wsb["wq"][k][:, m * 128:(m + 1) * 128],
                                xT[k][:, nt * 512:(nt + 1) * 512],
                                start=(k == 0), stop=(k == CT - 1),
                            )
                        nc.vector.tensor_scalar_add(
                            out=qT[m][:, nt * 512:(nt + 1) * 512],
                            in0=pq, scalar1=bqf[m],
                        )
                for jt in range(JT):
                    pv = psp.tile([128, 512], F32, tag="p")
                    for k in range(CT):
                        nc.tensor.matmul(
                            pv,
                            xT[k][:, jt * 128:(jt + 1) * 128],
                            wsb["wv"][k],
                            start=(k == 0), stop=(k == CT - 1),
                        )
                    nc.vector.tensor_add(out=v_sb[:, jt, :], in0=pv, in1=bvb)

            # ---- attention ----
            with (
                tc.tile_pool(name="ps_att", bufs=4, space="PSUM") as psa,
                tc.tile_pool(name="ps_s", bufs=2, space="PSUM") as pss,
                tc.tile_pool(name="ps_o", bufs=1, space="PSUM") as pso,
                tc.tile_pool(name="ps_den", bufs=1, space="PSUM") as psd,
            ):
                for ib in range(IB):
                    att_ps = [psa.tile([128, 512], F32, tag="att", name=f"att_ps{cs}")
                              for cs in range(CT)]
                    dacc = owork.tile([128, 512], F32, tag="dacc")
                    nc.vector.memset(dacc, 0.0)
                    for jt in range(JT):
                        s_ps = pss.tile([128, 512], F32, tag="s")
                        for k in range(CT):
                            nc.tensor.matmul(
                                s_ps,
                                kT[k][:, jt * 128:(jt + 1) * 128],
                                qT[k][:, ib * 512:(ib + 1) * 512],
                                start=(k == 0), stop=(k == CT - 1),
                            )
                        e_t = ework.tile([128, 512], BF16, tag="e")
                        nc.scalar.activation(
                            out=e_t, in_=s_ps,
                            func=mybir.ActivationFunctionType.Exp,
                            scale=SCALE,
                        )
                        for cs in range(CT):
                            nc.tensor.matmul(
                                att_ps[cs],
                                v_sb[:, jt, cs * 128:(cs + 1) * 128],
                                e_t,
                                start=(jt == 0), stop=(jt == JT - 1),
                            )
                        nc.vector.tensor_add(out=dacc, in0=dacc, in1=e_t)
                    # denominator: column sums of dacc (over j partitions)
                    den_ps = psd.tile([1, 512], F32, tag="den")
                    nc.tensor.matmul(den_ps, onesf_col, dacc, start=True, stop=True)
                    attT = []
                    for cs in range(CT):
                        t = attw.tile([128, 512], BF16, tag="attT", name=f"attT{cs}")
                        nc.vector.tensor_copy(out=t, in_=att_ps[cs])
                        attT.append(t)
                    den_sb = owork.tile([1, 512], F32, tag="den_sb")
                    nc.vector.tensor_copy(out=den_sb, in_=den_ps)
                    for it in range(4):
                        row0 = (ib * 4 + it) * 128
                        dT = pso.tile([128, 1], F32, tag="o",
                                      padded_shape=[128, 512], name=f"dT{it}")
                        nc.tensor.matmul(
                            dT, den_sb[0:1, it * 128:(it + 1) * 128], onef,
                            start=True, stop=True,
                        )
                        rden = rdenw.tile([128, 1], F32, tag="rden")
                        nc.vector.reciprocal(out=rden, in_=dT)
                        o_ps = pso.tile([128, 512], F32, tag="o", name=f"o_ps{it}")
                        for cs in range(CT):
                            nc.tensor.matmul(
                                o_ps,
                                attT[cs][:, it * 128:(it + 1) * 128],
                                wsb["wo"][cs],
                                start=(cs == 0), stop=(cs == CT - 1),
                            )
                        xr_t = xrw.tile([128, C], F32, tag="xr")
                        nc.sync.dma_start(
                            out=xr_t, in_=xr_e.ap()[row0:row0 + 128, :]
                        )
                        o_t = owork.tile([128, C], F32, tag="o")
                        nc.vector.scalar_tensor_tensor(
                            out=o_t, in0=o_ps, scalar=rden, in1=xr_t,
                            op0=mybir.AluOpType.mult, op1=mybir.AluOpType.add,
                        )
                        nc.sync.dma_start(
                            out=out_e.ap()[row0:row0 + 128, :], in_=o_t
                        )

    nc.compile()
    return nc


def _get_nc():
    if "nc" not in _CACHE:
        _CACHE["nc"] = _build()
    return _CACHE["nc"]


def kernel(**inputs) -> np.ndarray:
    x = np.asarray(inputs["x"], dtype=np.float32)          # [B,H,W,C]
    gn_scale = np.asarray(inputs["gn_scale"], np.float32)
    gn_bias = np.asarray(inputs["gn_bias"], np.float32)
    ws = {n: np.ascontiguousarray(
        np.asarray(inputs[n], np.float32).astype(ml_dtypes.bfloat16))
        for n in ("wq", "wk", "wv", "wo")}
    bs = {n: np.asarray(inputs[n], np.float32) for n in ("bq", "bk", "bv", "bo")}

    gmat = np.zeros((128, 128), np.float32)
    for g in range(128 // GS):
        gmat[g * GS:(g + 1) * GS, g * GS:(g + 1) * GS] = 1.0 / GS

    xf = x.reshape(B, N_TOK, C)
    in_maps = []
    for core in range(8):
        b, h = divmod(core, 2)
        own = xf[b, h * NQ:(h + 1) * NQ]          # [NQ, C] fp32
        other = xf[b, (1 - h) * NQ:(2 - h) * NQ]
        perm = np.concatenate([own, other], axis=0)        # own half first
        xt = np.ascontiguousarray(perm.T.astype(ml_dtypes.bfloat16))  # [C, N]
        xr = np.ascontiguousarray(own + bs["bo"][None, :])  # residual (+bo)
        in_maps.append({
            "xt": xt,
            "xr": xr,
            "wq": ws["wq"], "wk": ws["wk"], "wv": ws["wv"], "wo": ws["wo"],
            "bq": bs["bq"], "bk": bs["bk"], "bv": bs["bv"],
            "gsc": gn_scale, "gbi": gn_bias,
            "gmat": gmat,
        })

    nc = _get_nc()
    res = run_bass_kernel_spmd(nc, in_maps, core_ids=list(range(8)))

    out = np.empty((B, N_TOK, C), np.float32)
    for core in range(8):
        b, h = divmod(core, 2)
        out[b, h * NQ:(h + 1) * NQ] = res.results[core]["out"]
    return out.reshape(B, H, W, C)


# revision 18
# speedup vs baseline: 1.3326x; 1.0025x over previous
"""AttnBlock (GroupNorm + single-head self-attention + residual) on 8 TRN2
NeuronCores.

Reference computation (per image b of 4, tokens N=64*64=4096, C=512):
    hn  = GroupNorm(x)  (32 groups, eps 1e-6, affine)
    q,k,v = hn @ wq + bq, ...
    attn = softmax(q @ k.T / sqrt(C)); out = attn @ v
    y   = x + out @ wo + bo

Sharding: one NeuronCore per (image, half): core 2b+h computes attention
rows [h*2048, (h+1)*2048) of image b. Each core redundantly computes
GroupNorm stats and full-image K/V (cheap vs. cross-core collectives) and
its own 2048 query rows. No inter-core communication.

Per-core layout: everything feature-major ([C, tokens]) so every matmul
contraction sits on the partition axis; the final projection naturally
returns to row-major. The host pre-transposes/casts x to bf16
feature-major per core (shard prep) and passes the residual rows in f32.

Device pipeline:
  1. GroupNorm statistics via bn_stats/bn_aggr on the raw bf16 x
     (feature-major; per-channel over tokens, then group-combined with a
     block-diagonal averaging matmul).
  2. The normalization hn = x*A + B is FOLDED INTO THE QKV WEIGHTS:
     W' = A∘W (row scale), b' = b + B@W. The projections then consume the
     raw x tiles directly - no normalize pass on the critical path.
  3. Attention: scoresT = kT^T q (feature-major both sides), exp on the
     Scalar engine without max subtraction (scores provably in [-2,2] for
     unit-normalized inputs), softmax denominator accumulated on the
     Vector engine, attn@v and output projection on TensorE, with the
     1/denominator applied per query row in the epilogue (softmax
     normalization commutes with the linear attn@v and output proj).
Compute dtype: bf16 operands, f32 PSUM accumulation.
"""

import sys

if "/opt/trn_rl_repo" not in sys.path:
    sys.path.insert(0, "/opt/trn_rl_repo")

import numpy as np
import ml_dtypes

import concourse.bass as bass
import concourse.tile as tile
from concourse import bacc, mybir
from concourse.bass_utils import run_bass_kernel_spmd

F32 = mybir.dt.float32
BF16 = mybir.dt.bfloat16

B, H, W, C = 4, 64, 64, 512
N_TOK = H * W            # tokens per image
NQ = N_TOK // 2          # query rows per core
G = 32                   # groups
GS = C // G              # channels per group (16)
EPS = 1e-6
SCALE = float(C) ** -0.5
CT = C // 128            # channel tiles (4)
JT = N_TOK // 128        # token tiles (32)
IB = NQ // 512           # query i-blocks (4)

_CACHE = {}


def _build():
    nc = bacc.Bacc("TRN2", target_bir_lowering=False)

    xt_e = nc.dram_tensor("xt", [C, N_TOK], BF16, kind="ExternalInput")
    xr_e = nc.dram_tensor("xr", [NQ, C], F32, kind="ExternalInput")
    w_e = {
        n: nc.dram_tensor(n, [C, C], BF16, kind="ExternalInput")
        for n in ("wq", "wk", "wv", "wo")
    }
    b_e = {
        n: nc.dram_tensor(n, [C], F32, kind="ExternalInput")
        for n in ("bq", "bk", "bv")
    }
    gs_e = nc.dram_tensor("gsc", [C], F32, kind="ExternalInput")
    gb_e = nc.dram_tensor("gbi", [C], F32, kind="ExternalInput")
    gm_e = nc.dram_tensor("gmat", [128, 128], F32, kind="ExternalInput")
    out_e = nc.dram_tensor("out", [NQ, C], F32, kind="ExternalOutput")

    def col(e):  # [C] dram -> [C,1] view for partition-major loads
        return e.ap().rearrange("(a b) -> a b", b=1)

    with tile.TileContext(nc) as tc:
        with (
            tc.tile_pool(name="const", bufs=1) as const,
            tc.tile_pool(name="big", bufs=1) as big,
            tc.tile_pool(name="stat", bufs=1) as stat,
            tc.tile_pool(name="ework", bufs=4) as ework,
            tc.tile_pool(name="attw", bufs=8) as attw,
            tc.tile_pool(name="owork", bufs=4) as owork,
            tc.tile_pool(name="xrw", bufs=3) as xrw,
            tc.tile_pool(name="rdenw", bufs=8) as rdenw,
        ):
            # ---- x feature-major (bf16) ----
            # whole-tile contiguous DMAs (1 MB each; strided sub-chunk loads
            # are an order of magnitude slower), issued before the weight
            # loads so the stats-critical data arrives first
            xT = []
            for k in range(CT):
                t = big.tile([128, N_TOK], BF16, tag=f"xT{k}", name=f"xT{k}")
                for hh in range(2):
                    nc.sync.dma_start(
                        out=t[:, hh * 2048:(hh + 1) * 2048],
                        in_=xt_e.ap()[k * 128:(k + 1) * 128,
                                      hh * 2048:(hh + 1) * 2048],
                    )
                xT.append(t)

            # ---- weights / constants ----
            wsb = {}
            for n in ("wq", "wk", "wv", "wo"):
                wsb[n] = []
                for k in range(CT):
                    t = const.tile([128, C], BF16, tag=f"w_{n}_{k}")
                    nc.sync.dma_start(out=t, in_=w_e[n].ap()[k * 128:(k + 1) * 128, :])
                    wsb[n].append(t)
            bsb = {}
            for n in ("bq", "bk"):
                bsb[n] = []
                for m in range(CT):
                    t = const.tile([128, 1], F32, tag=f"b_{n}_{m}")
                    nc.sync.dma_start(out=t, in_=col(b_e[n])[m * 128:(m + 1) * 128, :])
                    bsb[n].append(t)
            bvb = const.tile([128, C], F32, tag="bvb")
            nc.sync.dma_start(
                out=bvb,
                in_=bass.AP(tensor=b_e["bv"], offset=0, ap=[[0, 128], [1, C]]),
            )
            gssb, gbsb = [], []
            for m in range(CT):
                t = const.tile([128, 1], F32, tag=f"gs_{m}")
                nc.sync.dma_start(out=t, in_=col(gs_e)[m * 128:(m + 1) * 128, :])
                gssb.append(t)
                t = const.tile([128, 1], F32, tag=f"gb_{m}")
                nc.sync.dma_start(out=t, in_=col(gb_e)[m * 128:(m + 1) * 128, :])
                gbsb.append(t)
            gm_sb = const.tile([128, 128], F32, tag="gmat")
            nc.sync.dma_start(out=gm_sb, in_=gm_e.ap())
            ones_bcol = const.tile([1, 128], F32, tag="ones_bcol")
            nc.vector.memset(ones_bcol, 1.0)
            onef = const.tile([1, 1], F32, tag="onef")
            nc.vector.memset(onef, 1.0)
            onesf_col = const.tile([128, 1], F32, tag="onesf_col")
            nc.vector.memset(onesf_col, 1.0)
            epst = const.tile([128, 1], F32, tag="epst")
            nc.vector.memset(epst, EPS)

            # ---- GroupNorm stats + weight folding ----
            with tc.tile_pool(name="ps_misc", bufs=2, space="PSUM") as psm:
                Af, Bbf = [], []   # A (f32 [128,1]); B cast to bf16 for matmuls
                for k in range(CT):
                    stats = stat.tile([128, 8, 6], F32, tag=f"st{k}")
                    for ch in range(8):
                        nc.vector.bn_stats(
                            out=stats[:, ch, :],
                            in_=xT[k][:, ch * 512:(ch + 1) * 512],
                        )
                    mv = stat.tile([128, 2], F32, tag=f"mv{k}")
                    nc.vector.bn_aggr(out=mv, in_=stats)
                    # sm = (mean, var + mean^2) per channel
                    sm = stat.tile([128, 2], F32, tag=f"sm{k}")
                    nc.vector.tensor_copy(out=sm[:, 0:1], in_=mv[:, 0:1])
                    nc.vector.tensor_mul(out=sm[:, 1:2], in0=mv[:, 0:1], in1=mv[:, 0:1])
                    nc.vector.tensor_add(out=sm[:, 1:2], in0=sm[:, 1:2], in1=mv[:, 1:2])
                    # group-average via block-diagonal (1/GS) matrix
                    gps = psm.tile([128, 2], F32, tag="gps")
                    nc.tensor.matmul(gps, gm_sb, sm, start=True, stop=True)
                    gsb = stat.tile([128, 2], F32, tag=f"gsb{k}")
                    nc.vector.tensor_copy(out=gsb, in_=gps)
                    # var_g = E_g[x^2]-mean_g^2; A = rstd*scale; B = bias-mean_g*A
                    msq = stat.tile([128, 1], F32, tag=f"msq{k}")
                    nc.vector.tensor_mul(out=msq, in0=gsb[:, 0:1], in1=gsb[:, 0:1])
                    varg = stat.tile([128, 1], F32, tag=f"vg{k}")
                    nc.vector.tensor_sub(out=varg, in0=gsb[:, 1:2], in1=msq)
                    sd = stat.tile([128, 1], F32, tag=f"sd{k}")
                    nc.scalar.activation(
                        out=sd, in_=varg,
                        func=mybir.ActivationFunctionType.Sqrt,
                        bias=epst, scale=1.0,
                    )
                    rstd = stat.tile([128, 1], F32, tag=f"rs{k}")
                    nc.vector.reciprocal(out=rstd, in_=sd)
                    At = stat.tile([128, 1], F32, tag=f"A{k}")
                    nc.vector.tensor_mul(out=At, in0=rstd, in1=gssb[k])
                    mA = stat.tile([128, 1], F32, tag=f"mA{k}")
                    nc.vector.tensor_mul(out=mA, in0=gsb[:, 0:1], in1=At)
                    Bt = stat.tile([128, 1], F32, tag=f"B{k}")
                    nc.vector.tensor_sub(out=Bt, in0=gbsb[k], in1=mA)
                    Bb = stat.tile([128, 1], BF16, tag=f"Bb{k}")
                    nc.vector.tensor_copy(out=Bb, in_=Bt)
                    Af.append(At)
                    Bbf.append(Bb)

                # fold normalization into weights:
                #   W' = A (row) ∘ W ;  b' = b + B @ W
                # B@W matmuls read the ORIGINAL W (Tile orders them before the
                # in-place row scale below via WAR deps).
                badj = {}
                for n in ("wq", "wk", "wv"):
                    pb = psm.tile([1, 512], F32, tag="pb", name=f"pb_{n}")
                    for k in range(CT):
                        nc.tensor.matmul(
                            pb, Bbf[k], wsb[n][k],
                            start=(k == 0), stop=(k == CT - 1),
                        )
                    bs_ = stat.tile([1, 512], F32, tag=f"badj_{n}")
                    nc.vector.tensor_copy(out=bs_, in_=pb)
                    badj[n] = bs_
                # row-scale the weights on the Scalar engine (idle here;
                # keeps DVE free for the stats chain)
                for n in ("wq", "wk", "wv"):
                    for k in range(CT):
                        nc.scalar.activation(
                            out=wsb[n][k], in_=wsb[n][k],
                            func=mybir.ActivationFunctionType.Copy,
                            scale=Af[k],
                        )
                # transpose b' pieces to per-partition layout for q/k;
                # build broadcast bias for v.
                bqf, bkf = [], []
                for n, dst in (("wq", bqf), ("wk", bkf)):
                    for m in range(CT):
                        pt = psm.tile([128, 1], F32, tag="pt", name=f"pt_{n}{m}")
                        nc.tensor.matmul(
                            pt, badj[n][0:1, m * 128:(m + 1) * 128], onef,
                            start=True, stop=True,
                        )
                        bf = stat.tile([128, 1], F32, tag=f"bf_{n}{m}")
                        base = bsb["bq"][m] if n == "wq" else bsb["bk"][m]
                        nc.vector.tensor_add(out=bf, in0=pt, in1=base)
                        dst.append(bf)
                pvb = psm.tile([128, 512], F32, tag="pvb")
                nc.tensor.matmul(pvb, ones_bcol, badj["wv"], start=True, stop=True)
                nc.vector.tensor_add(out=bvb, in0=pvb, in1=bvb)

            # ---- projections (raw x in, folded weights) ----
            kT = [big.tile([128, N_TOK], BF16, tag=f"kT{m}", name=f"kT{m}")
                  for m in range(CT)]
            qT = [big.tile([128, NQ], BF16, tag=f"qT{m}", name=f"qT{m}")
                  for m in range(CT)]
            v_sb = big.tile([128, JT, C], BF16, tag="v")
            with tc.tile_pool(name="ps_proj", bufs=6, space="PSUM") as psp:
                for m in range(CT):
                    for nt in range(N_TOK // 512):
                        pk = psp.tile([128, 512], F32, tag="p")
                        for k in range(CT):
                            nc.tensor.matmul(
                                pk,
                                wsb["wk"][k][:, m * 128:(m + 1) * 128],
                                xT[k][:, nt * 512:(nt + 1) * 512],
                                start=(k == 0), stop=(k == CT - 1),
                            )
                        nc.vector.tensor_scalar_add(
                            out=kT[m][:, nt * 512:(nt + 1) * 512],
                            in0=pk, scalar1=bkf[m],
                        )
                    for nt in range(NQ // 512):
                        pq = psp.tile([128, 512], F32, tag="p")
                        for k in range(CT):
                            nc.tensor.matmul(
                                pq,
                                wsb["wq"][k][:, m * 128:(m + 1) * 128],
                                xT[k][:, nt * 512:(nt + 1) * 512],
                                start=(k == 0), stop=(k == CT - 1),
                            )
                        nc.vector.tensor_scalar_add(
                            out=qT[m][:, nt * 512:(nt + 1) * 512],
                            in0=pq, scalar1=bqf[m],
                        )
                for jt in range(JT):
                    pv = psp.tile([128, 512], F32, tag="p")
                    for k in range(CT):
                        nc.tensor.matmul(
                            pv,
                            xT[k][:, jt * 128:(jt + 1) * 128],
                            wsb["wv"][k],
                            start=(k == 0), stop=(k == CT - 1),
                        )
                    nc.vector.tensor_add(out=v_sb[:, jt, :], in0=pv, in1=bvb)

            # ---- attention ----
            with (
                tc.tile_pool(name="ps_att", bufs=4, space="PSUM") as psa,
                tc.tile_pool(name="ps_s", bufs=2, space="PSUM") as pss,
                tc.tile_pool(name="ps_o", bufs=1, space="PSUM") as pso,
                tc.tile_pool(name="ps_den", bufs=1, space="PSUM") as psd,
            ):
                for ib in range(IB):
                    att_ps = [psa.tile([128, 512], F32, tag="att", name=f"att_ps{cs}")
                              for cs in range(CT)]
                    dacc = owork.tile([128, 512], F32, tag="dacc")
                    nc.vector.memset(dacc, 0.0)
                    for jt in range(JT):
                        s_ps = pss.tile([128, 512], F32, tag="s")
                        for k in range(CT):
                            nc.tensor.matmul(
                                s_ps,
                                kT[k][:, jt * 128:(jt + 1) * 128],
                                qT[k][:, ib * 512:(ib + 1) * 512],
                                start=(k == 0), stop=(k == CT - 1),
                            )
                        e_t = ework.tile([128, 512], BF16, tag="e")
                        nc.scalar.activation(
                            out=e_t, in_=s_ps,
                            func=mybir.ActivationFunctionType.Exp,
                            scale=SCALE,
                        )
                        for cs in range(CT):
                            nc.tensor.matmul(
                                att_ps[cs],
                                v_sb[:, jt, cs * 128:(cs + 1) * 128],
                                e_t,
                                start=(jt == 0), stop=(jt == JT - 1),
                            )
                        nc.vector.tensor_add(out=dacc, in0=dacc, in1=e_t)
                    # denominator: column sums of dacc (over j partitions)
                    den_ps = psd.tile([1, 512], F32, tag="den")
                    nc.tensor.matmul(den_ps, onesf_col, dacc, start=True, stop=True)
                    attT = []
                    for cs in range(CT):
                        t = attw.tile([128, 512], BF16, tag="attT", name=f"attT{cs}")
                        nc.vector.tensor_copy(out=t, in_=att_ps[cs])
                        attT.append(t)
                    den_sb = owork.tile([1, 512], F32, tag="den_sb")
                    nc.vector.tensor_copy(out=den_sb, in_=den_ps)
                    for it in range(4):
                        row0 = (ib * 4 + it) * 128
                        dT = pso.tile([128, 1], F32, tag="o",
                                      padded_shape=[128, 512], name=f"dT{it}")
                        nc.tensor.matmul(
                            dT, den_sb[0:1, it * 128:(it + 1) * 128], onef,
                            start=True, stop=True,
                        )
                        rden = rdenw.tile([128, 1], F32, tag="rden")
                        nc.vector.reciprocal(out=rden, in_=dT)
                        o_ps = pso.tile([128, 512], F32, tag="o", name=f"o_ps{it}")
                        for cs in range(CT):
                            nc.tensor.matmul(
                                o_ps,
                                attT[cs][:, it * 128:(it + 1) * 128],
                                wsb["wo"][cs],
                                start=(cs == 0), stop=(cs == CT - 1),
                            )
                        xr_t = xrw.tile([128, C], F32, tag="xr")
                        nc.sync.dma_start(
                            out=xr_t, in_=xr_e.ap()[row0:row0 + 128, :]
                        )
                        o_t = owork.tile([128, C], F32, tag="o")
                        nc.vector.scalar_tensor_tensor(
                            out=o_t, in0=o_ps, scalar=rden, in1=xr_t,
                            op0=mybir.AluOpType.mult, op1=mybir.AluOpType.add,
                        )
                        nc.sync.dma_start(
                            out=out_e.ap()[row0:row0 + 128, :], in_=o_t
                        )

    nc.compile()
    return nc


def _get_nc():
    if "nc" not in _CACHE:
        _CACHE["nc"] = _build()
    return _CACHE["nc"]


def kernel(**inputs) -> np.ndarray:
    x = np.asarray(inputs["x"], dtype=np.float32)          # [B,H,W,C]
    gn_scale = np.asarray(inputs["gn_scale"], np.float32)
    gn_bias = np.asarray(inputs["gn_bias"], np.float32)
    ws = {n: np.ascontiguousarray(
        np.asarray(inputs[n], np.float32).astype(ml_dtypes.bfloat16))
        for n in ("wq", "wk", "wv", "wo")}
    bs = {n: np.asarray(inputs[n], np.float32) for n in ("bq", "bk", "bv", "bo")}

    gmat = np.zeros((128, 128), np.float32)
    for g in range(128 // GS):
        gmat[g * GS:(g + 1) * GS, g * GS:(g + 1) * GS] = 1.0 / GS

    xf = x.reshape(B, N_TOK, C)
    in_maps = []
    for core in range(8):
        b, h = divmod(core, 2)
        own = xf[b, h * NQ:(h + 1) * NQ]          # [NQ, C] fp32
        other = xf[b, (1 - h) * NQ:(2 - h) * NQ]
        perm = np.concatenate([own, other], axis=0)        # own half first
        xt = np.ascontiguousarray(perm.T.astype(ml_dtypes.bfloat16))  # [C, N]
        xr = np.ascontiguousarray(own + bs["bo"][None, :])  # residual (+bo)
        in_maps.append({
            "xt": xt,
            "xr": xr,
            "wq": ws["wq"], "wk": ws["wk"], "wv": ws["wv"], "wo": ws["wo"],
            "bq": bs["bq"], "bk": bs["bk"], "bv": bs["bv"],
            "gsc": gn_scale, "gbi": gn_bias,
            "gmat": gmat,
        })

    nc = _get_nc()
    res = run_bass_kernel_spmd(nc, in_maps, core_ids=list(range(8)))

    out = np.empty((B, N_TOK, C), np.float32)
    for core in range(8):
        b, h = divmod(core, 2)
        out[b, h * NQ:(h + 1) * NQ] = res.results[core]["out"]
    return out.reshape(B, H, W, C)
